# revision 7
# baseline (speedup 1.0000x reference)
"""GATv2Conv message-passing kernel for 8 Trainium2 NeuronCores.

Strategy (receiver-sharded, padded-grid, no collectives):
- Nodes are sorted by in-degree and dealt round-robin to the 8 cores, so each
  core owns ~12.5k receiver nodes with a balanced edge count, and consecutive
  128-node tiles have near-uniform degree (padding ratio ~1.02).
- Each core computes the full sender projection table s_proj = [x|1] @ [Ws;bs]
  on-device into an HBM scratch table (replicated work), and its local receiver
  projection r_proj into SBUF.
- Per 128-node tile, sender rows are fetched with per-partition indirect DMAs
  (one 128-row gather per in-edge slot k), then the whole GATv2 edge math
  (mish, logits, masked softmax without max-subtraction -- logits are O(5) for
  this input distribution -- and the weighted aggregation) runs as dense
  DVE/ACT ops over the [128, D_t*64] grid. Output rows stream back contiguous;
  the host inverse-permutes.
"""

import numpy as np

import concourse.bass as bass
import concourse.bacc as bacc
import concourse.mybir as mybir
import concourse.tile as tile
from concourse.bass import IndirectOffsetOnAxis
from concourse.bass_utils import run_bass_kernel_spmd

F32 = mybir.dt.float32
I32 = mybir.dt.int32

N_NODES = 100000
N_EDGES = 1600000
F = 64
H = 4
HD = 16
NC_CORES = 8


def _host_prep(x, Ws, bs, Wr, br, aw, ab, senders, receivers):
    """Pure index/layout work: shard nodes+edges, build grid slot arrays."""
    N = x.shape[0]
    deg = np.bincount(receivers, minlength=N)
    order = np.argsort(deg, kind="stable").astype(np.int64)  # rank -> node
    inv_order = np.empty(N, dtype=np.int64)
    inv_order[order] = np.arange(N)

    rows_per_core = -(-N // NC_CORES)          # 12500
    tiles = -(-rows_per_core // 128)           # 98
    rows_pad = tiles * 128                     # 12544

    # per-tile max degree over the 1024-rank window (common across cores)
    d_pad = np.zeros(tiles * 1024, dtype=np.int64)
    d_pad[: N] = deg[order]
    D_t = d_pad.reshape(tiles, 1024).max(axis=1)
    D_t = np.maximum(D_t, 1)
    OFF = np.concatenate([[0], np.cumsum(D_t)]).astype(np.int64)
    S = int(OFF[-1])

    # edge -> (core, row, k)
    erank = inv_order[receivers]
    e_sort = np.argsort(erank, kind="stable")
    er_sorted = erank[e_sort]
    s_sorted = senders[e_sort]
    # k = position within each receiver's edge list
    grp_start = np.searchsorted(er_sorted, np.arange(N))
    k_all = np.arange(len(er_sorted)) - grp_start[er_sorted]

    core_e = er_sorted % NC_CORES
    row_e = er_sorted // NC_CORES
    t_e = row_e // 128
    p_e = row_e % 128
    col_e = OFF[t_e] + k_all

    idx_arr = np.zeros((NC_CORES, 128, S), dtype=np.int32)
    mask_arr = np.zeros((NC_CORES, 128, S), dtype=np.float32)
    idx_arr[core_e, p_e, col_e] = s_sorted.astype(np.int32)
    mask_arr[core_e, p_e, col_e] = 1.0

    # x^T padded + ones row, shared across cores
    n_grp = -(-N // 512)
    n_tab = n_grp * 512
    xT_aug = np.zeros((F + 1, n_tab), dtype=np.float32)
    xT_aug[:F, :N] = x.T
    xT_aug[F, :] = 1.0

    # per-core local x^T (+ones)
    xlT = np.zeros((NC_CORES, F + 1, rows_pad), dtype=np.float32)
    for c in range(NC_CORES):
        rows = order[c::NC_CORES]          # ranks c, c+8, ... in ascending rank
        xlT[c, :F, : len(rows)] = x[rows].T
        xlT[c, F, :] = 1.0

    Wsb = np.concatenate([Ws.reshape(F, F), bs.reshape(1, F)], axis=0).astype(np.float32)
    Wrb = np.concatenate([Wr.reshape(F, F), br.reshape(1, F)], axis=0).astype(np.float32)
    aw_rep = np.tile(np.asarray(aw, np.float32).reshape(1, HD), (1, H)).reshape(1, F)
    awb = np.tile(aw_rep, (128, 1)).astype(np.float32)

    meta = dict(
        D_t=D_t.astype(int).tolist(),
        OFF=OFF.astype(int).tolist(),
        S=S,
        tiles=tiles,
        rows_pad=rows_pad,
        n_tab=n_tab,
        n_grp=n_grp,
        order=order,
        ab=float(np.asarray(ab).reshape(-1)[0]),
    )
    ins = dict(xT=xT_aug, xlT=xlT, Wsb=Wsb, Wrb=Wrb, awb=awb,
               idx=idx_arr, mask=mask_arr)
    return ins, meta


def _build_program(meta):
    D_t, OFF, S = meta["D_t"], meta["OFF"], meta["S"]
    tiles, rows_pad, n_tab, n_grp = (
        meta["tiles"], meta["rows_pad"], meta["n_tab"], meta["n_grp"])
    ab = meta["ab"]

    nc = bacc.Bacc()
    xT = nc.declare_dram_parameter("xT", [F + 1, n_tab], F32, isOutput=False)
    xlT = nc.declare_dram_parameter("xlT", [F + 1, rows_pad], F32, isOutput=False)
    Wsb = nc.declare_dram_parameter("Wsb", [F + 1, F], F32, isOutput=False)
    Wrb = nc.declare_dram_parameter("Wrb", [F + 1, F], F32, isOutput=False)
    awb = nc.declare_dram_parameter("awb", [128, F], F32, isOutput=False)
    idxp = nc.declare_dram_parameter("idx", [128, S], I32, isOutput=False)
    maskp = nc.declare_dram_parameter("mask", [128, S], F32, isOutput=False)
    outp = nc.declare_dram_parameter("out", [rows_pad, F], F32, isOutput=True)

    AT = mybir.ActivationFunctionType
    ALU = mybir.AluOpType

    with tile.TileContext(nc) as tc:
        with (
            tc.tile_pool(name="dram", bufs=1, space="DRAM") as dpool,
            tc.tile_pool(name="consts", bufs=1) as cpool,
            tc.tile_pool(name="xload", bufs=3) as xpool,
            tc.tile_pool(name="pse", bufs=3) as pse,
            tc.tile_pool(name="pz", bufs=2) as pz,
            tc.tile_pool(name="pa", bufs=2) as pa,
            tc.tile_pool(name="pb", bufs=2) as pb,
            tc.tile_pool(name="small", bufs=3) as spool,
            tc.tile_pool(name="psum", bufs=2, space="PSUM") as ppool,
        ):
            table = dpool.tile([n_tab, F], F32)

            wsb_sb = cpool.tile([F + 1, F], F32)
            nc.sync.dma_start(out=wsb_sb[:], in_=Wsb[:])
            wrb_sb = cpool.tile([F + 1, F], F32)
            nc.sync.dma_start(out=wrb_sb[:], in_=Wrb[:])
            awb_sb = cpool.tile([128, F], F32)
            nc.sync.dma_start(out=awb_sb[:], in_=awb[:])
            idx_sb = cpool.tile([128, S], I32)
            nc.sync.dma_start(out=idx_sb[:], in_=idxp[:])
            mask_sb = cpool.tile([128, S], F32)
            nc.sync.dma_start(out=mask_sb[:], in_=maskp[:])
            r_sb = cpool.tile([128, tiles * F], F32)

            # phase 1a: r_proj for local nodes, resident in SBUF
            for t in range(tiles):
                xt = xpool.tile([F + 1, 128], F32, tag="xl")
                nc.sync.dma_start(out=xt[:], in_=xlT[:, t * 128:(t + 1) * 128])
                ps = ppool.tile([128, F], F32, tag="psr")
                nc.tensor.matmul(ps[:], lhsT=xt[:], rhs=wrb_sb[:],
                                 start=True, stop=True)
                nc.scalar.copy(r_sb[:, t * F:(t + 1) * F], ps[:])

            # phase 1b: s_proj table in HBM
            for g in range(n_grp):
                xg = xpool.tile([F + 1, 512], F32, tag="xg")
                nc.sync.dma_start(out=xg[:], in_=xT[:, g * 512:(g + 1) * 512])
                ps = ppool.tile([128, 4 * F], F32, tag="pss")
                for j in range(4):
                    nc.tensor.matmul(
                        ps[:, j * F:(j + 1) * F],
                        lhsT=xg[:, j * 128:(j + 1) * 128],
                        rhs=wsb_sb[:], start=True, stop=True)
                sg = xpool.tile([128, 4 * F], F32, tag="sg")
                nc.vector.tensor_copy(sg[:], ps[:])
                nc.sync.dma_start(
                    out=table[g * 512:(g + 1) * 512, :].rearrange(
                        "(j p) c -> p j c", p=128),
                    in_=sg[:].rearrange("p (j c) -> p j c", j=4))

            # phase 2: per-tile gather + edge math
            for t in range(tiles):
                Dt = D_t[t]
                off = OFF[t]
                KC = Dt * F
                se = pse.tile([128, KC], F32, tag="se")
                for k in range(Dt):
                    nc.gpsimd.indirect_dma_start(
                        out=se[:, k * F:(k + 1) * F],
                        out_offset=None,
                        in_=table[:],
                        in_offset=IndirectOffsetOnAxis(
                            ap=idx_sb[:, off + k:off + k + 1], axis=0),
                    )
                re_b = r_sb[:, t * F:(t + 1) * F][:, None, :].to_broadcast(
                    [128, Dt, F])
                z = pz.tile([128, KC], F32, tag="z")
                nc.vector.tensor_tensor(
                    out=z[:].rearrange("p (k c) -> p k c", c=F),
                    in0=se[:].rearrange("p (k c) -> p k c", c=F),
                    in1=re_b, op=ALU.add)
                # mish(z) = z * tanh(softplus(z)) = z * (1 - 2/((e^z+1)^2+1))
                # (no Mish LUT in this build; Exp+Square share one table set)
                et = pa.tile([128, KC], F32, tag="A")
                nc.scalar.activation(et[:], z[:], AT.Exp)
                q = pb.tile([128, KC], F32, tag="B")
                nc.scalar.activation(q[:], et[:], AT.Square, bias=1.0)
                den_m = pa.tile([128, KC], F32, tag="A")
                nc.vector.tensor_scalar_add(den_m[:], in0=q[:], scalar1=1.0)
                rcp_m = pb.tile([128, KC], F32, tag="B")
                nc.vector.reciprocal(rcp_m[:], den_m[:])
                zr = pa.tile([128, KC], F32, tag="A")
                nc.vector.tensor_tensor(out=zr[:], in0=z[:], in1=rcp_m[:],
                                        op=ALU.mult)
                m = pb.tile([128, KC], F32, tag="B")
                nc.vector.scalar_tensor_tensor(
                    out=m[:], in0=zr[:], scalar=-2.0, in1=z[:],
                    op0=ALU.mult, op1=ALU.add)
                aw_b = awb_sb[:][:, None, :].to_broadcast([128, Dt, F])
                mw = pa.tile([128, KC], F32, tag="A")
                nc.vector.tensor_tensor(
                    out=mw[:].rearrange("p (k c) -> p k c", c=F),
                    in0=m[:].rearrange("p (k c) -> p k c", c=F),
                    in1=aw_b, op=ALU.mult)
                logits = spool.tile([128, Dt * H], F32, tag="logits")
                nc.vector.tensor_reduce(
                    out=logits[:],
                    in_=mw[:].rearrange("p (k h d) -> p k h d", h=H, d=HD),
                    axis=mybir.AxisListType.X, op=ALU.add)
                # ab cancels in the softmax (constant shift) -- skip it
                ex = spool.tile([128, Dt * H], F32, tag="ex")
                nc.scalar.activation(ex[:], logits[:], AT.Exp)
                exm = spool.tile([128, Dt * H], F32, tag="exm")
                mask_b = mask_sb[:, off:off + Dt][:, :, None].to_broadcast(
                    [128, Dt, H])
                nc.vector.tensor_tensor(
                    out=exm[:].rearrange("p (k h) -> p k h", h=H),
                    in0=ex[:].rearrange("p (k h) -> p k h", h=H),
                    in1=mask_b, op=ALU.mult)
                den = spool.tile([128, H], F32, tag="den")
                nc.vector.tensor_reduce(
                    out=den[:],
                    in_=exm[:].rearrange("p (k h) -> p h k", h=H),
                    axis=mybir.AxisListType.X, op=ALU.add)
                rec = spool.tile([128, H], F32, tag="rec")
                nc.vector.reciprocal(rec[:], den[:])
                wse = pb.tile([128, KC], F32, tag="B")
                exm_b = exm[:].rearrange(
                    "p (k h) -> p k h", h=H)[:, :, :, None].to_broadcast(
                    [128, Dt, H, HD])
                nc.vector.tensor_tensor(
                    out=wse[:].rearrange("p (k h d) -> p k h d", h=H, d=HD),
                    in0=se[:].rearrange("p (k h d) -> p k h d", h=H, d=HD),
                    in1=exm_b, op=ALU.mult)
                num = spool.tile([128, F], F32, tag="num")
                nc.vector.tensor_reduce(
                    out=num[:],
                    in_=wse[:].rearrange("p (k c) -> p c k", c=F),
                    axis=mybir.AxisListType.X, op=ALU.add)
                ot = spool.tile([128, F], F32, tag="ot")
                rec_b = rec[:][:, :, None].to_broadcast([128, H, HD])
                nc.vector.tensor_tensor(
                    out=ot[:].rearrange("p (h d) -> p h d", h=H),
                    in0=num[:].rearrange("p (h d) -> p h d", h=H),
                    in1=rec_b, op=ALU.mult)
                nc.sync.dma_start(out=outp[t * 128:(t + 1) * 128, :], in_=ot[:])

    return nc


def kernel(x, Ws, bs, Wr, br, aw, ab, senders, receivers):
    x = np.asarray(x, np.float32)
    senders = np.asarray(senders, np.int32)
    receivers = np.asarray(receivers, np.int32)
    ins, meta = _host_prep(x, np.asarray(Ws), np.asarray(bs), np.asarray(Wr),
                           np.asarray(br), np.asarray(aw), np.asarray(ab),
                           senders, receivers)
    nc = _build_program(meta)
    if not nc.is_finalized():
        nc.finalize()
    in_maps = []
    for c in range(NC_CORES):
        in_maps.append({
            "xT": ins["xT"],
            "xlT": ins["xlT"][c],
            "Wsb": ins["Wsb"],
            "Wrb": ins["Wrb"],
            "awb": ins["awb"],
            "idx": ins["idx"][c],
            "mask": ins["mask"][c],
        })
    res = run_bass_kernel_spmd(nc, in_maps, core_ids=list(range(NC_CORES)))
    N = x.shape[0]
    order = meta["order"]
    out_full = np.zeros((N, F), dtype=np.float32)
    rows_per_core = -(-N // NC_CORES)
    for c in range(NC_CORES):
        rows = order[c::NC_CORES]
        out_full[rows] = res.results[c]["out"][: len(rows)]
    return out_full


# revision 13
# speedup vs baseline: 3.1574x; 3.1574x over previous
"""GATv2Conv message-passing kernel for 8 Trainium2 NeuronCores.

Strategy (receiver-sharded, padded-grid, no collectives):
- Nodes are sorted by in-degree and dealt round-robin to the 8 cores, so each
  core owns ~12.5k receiver nodes with a balanced edge count, and consecutive
  128-node tiles have near-uniform degree (padding ratio ~1.02).
- Each core computes the full sender projection table s_proj = [x|1] @ [Ws;bs]
  on-device into an HBM scratch table (replicated work), and its local receiver
  projection r_proj into SBUF.
- Per 128-node tile, sender rows are fetched with per-partition indirect DMAs
  (one 128-row gather per in-edge slot k), then the whole GATv2 edge math
  (mish, logits, masked softmax without max-subtraction -- logits are O(5) for
  this input distribution -- and the weighted aggregation) runs as dense
  DVE/ACT ops over the [128, D_t*64] grid. Output rows stream back contiguous;
  the host inverse-permutes.
"""

import numpy as np

import concourse.bass as bass
import concourse.bacc as bacc
import concourse.mybir as mybir
import concourse.tile as tile
from concourse.bass import IndirectOffsetOnAxis
from concourse.bass_utils import run_bass_kernel_spmd

F32 = mybir.dt.float32
F16 = mybir.dt.float16
I32 = mybir.dt.int32

N_NODES = 100000
N_EDGES = 1600000
F = 64
H = 4
HD = 16
NC_CORES = 8


def _host_prep(x, Ws, bs, Wr, br, aw, ab, senders, receivers):
    """Pure index/layout work: shard nodes+edges, build grid slot arrays."""
    N = x.shape[0]
    deg = np.bincount(receivers, minlength=N)
    order = np.argsort(deg, kind="stable").astype(np.int64)  # rank -> node
    inv_order = np.empty(N, dtype=np.int64)
    inv_order[order] = np.arange(N)

    rows_per_core = -(-N // NC_CORES)          # 12500
    tiles = -(-rows_per_core // 128)           # 98
    rows_pad = tiles * 128                     # 12544

    # per-tile max degree over the 1024-rank window (common across cores)
    d_pad = np.zeros(tiles * 1024, dtype=np.int64)
    d_pad[: N] = deg[order]
    D_t = d_pad.reshape(tiles, 1024).max(axis=1)
    D_t = np.maximum(D_t, 1)
    OFF = np.concatenate([[0], np.cumsum(D_t)]).astype(np.int64)
    S = int(OFF[-1])

    # edge -> (core, row, k)
    erank = inv_order[receivers]
    e_sort = np.argsort(erank, kind="stable")
    er_sorted = erank[e_sort]
    s_sorted = senders[e_sort]
    # k = position within each receiver's edge list
    grp_start = np.searchsorted(er_sorted, np.arange(N))
    k_all = np.arange(len(er_sorted)) - grp_start[er_sorted]

    core_e = er_sorted % NC_CORES
    row_e = er_sorted // NC_CORES
    t_e = row_e // 128
    p_e = row_e % 128
    col_e = OFF[t_e] + k_all

    # senders are looked up in a permuted table: node n = g*512 + j*128 + p
    # lands at table row g*512 + p*4 + j (lets phase-1b write 1KB-contiguous
    # runs per partition instead of 256B descriptors)
    if TAU_WRITE:
        g_n = s_sorted // 512
        rem = s_sorted % 512
        j_n = rem // 128
        p_n = rem % 128
        tau = (g_n * 512 + p_n * 4 + j_n).astype(np.int32)
    else:
        tau = s_sorted.astype(np.int32)
    idx_arr = np.zeros((NC_CORES, 128, S), dtype=np.int32)
    mask_arr = np.zeros((NC_CORES, 128, S), dtype=np.float32)
    idx_arr[core_e, p_e, col_e] = tau
    mask_arr[core_e, p_e, col_e] = 1.0

    # x^T padded + ones row, shared across cores
    n_grp = -(-N // 512)
    n_tab = n_grp * 512
    xT_aug = np.zeros((F + 1, n_tab), dtype=np.float32)
    xT_aug[:F, :N] = x.T
    xT_aug[F, :] = 1.0

    # per-core local x^T (+ones)
    xlT = np.zeros((NC_CORES, F + 1, rows_pad), dtype=np.float32)
    for c in range(NC_CORES):
        rows = order[c::NC_CORES]          # ranks c, c+8, ... in ascending rank
        xlT[c, :F, : len(rows)] = x[rows].T
        xlT[c, F, :] = 1.0

    Wsb = np.concatenate([Ws.reshape(F, F), bs.reshape(1, F)], axis=0).astype(np.float32)
    Wrb = np.concatenate([Wr.reshape(F, F), br.reshape(1, F)], axis=0).astype(np.float32)
    aw_rep = np.tile(np.asarray(aw, np.float32).reshape(1, HD), (1, H)).reshape(1, F)
    awb = np.tile(aw_rep, (128, 1)).astype(np.float32)

    meta = dict(
        D_t=D_t.astype(int).tolist(),
        OFF=OFF.astype(int).tolist(),
        S=S,
        tiles=tiles,
        rows_pad=rows_pad,
        n_tab=n_tab,
        n_grp=n_grp,
        order=order,
        ab=float(np.asarray(ab).reshape(-1)[0]),
    )
    ins = dict(xT=xT_aug, xlT=xlT, Wsb=Wsb, Wrb=Wrb, awb=awb,
               idx=idx_arr, mask=mask_arr)
    return ins, meta


VARIANT = "full"  # full | gather_only | compute_only | phase1_only | empty
FP16_MISH = True  # run the mish/logits chain in fp16 (2x DVE modes)
TAU_WRITE = True  # permuted table rows for 1KB-contiguous phase-1b writes


def _build_program(meta):
    D_t, OFF, S = meta["D_t"], meta["OFF"], meta["S"]
    tiles, rows_pad, n_tab, n_grp = (
        meta["tiles"], meta["rows_pad"], meta["n_tab"], meta["n_grp"])
    ab = meta["ab"]

    nc = bacc.Bacc()
    xT = nc.declare_dram_parameter("xT", [F + 1, n_tab], F32, isOutput=False)
    xlT = nc.declare_dram_parameter("xlT", [F + 1, rows_pad], F32, isOutput=False)
    Wsb = nc.declare_dram_parameter("Wsb", [F + 1, F], F32, isOutput=False)
    Wrb = nc.declare_dram_parameter("Wrb", [F + 1, F], F32, isOutput=False)
    awb = nc.declare_dram_parameter("awb", [128, F], F32, isOutput=False)
    idxp = nc.declare_dram_parameter("idx", [128, S], I32, isOutput=False)
    maskp = nc.declare_dram_parameter("mask", [128, S], F32, isOutput=False)
    outp = nc.declare_dram_parameter("out", [rows_pad, F], F32, isOutput=True)

    AT = mybir.ActivationFunctionType
    ALU = mybir.AluOpType

    with tile.TileContext(nc) as tc:
        with (
            tc.tile_pool(name="dram", bufs=1, space="DRAM") as dpool,
            tc.tile_pool(name="consts", bufs=1) as cpool,
            tc.tile_pool(name="xload", bufs=3) as xpool,
            tc.tile_pool(name="pse", bufs=3) as pse,
            tc.tile_pool(name="pz", bufs=2) as pz,
            tc.tile_pool(name="pa", bufs=2) as pa,
            tc.tile_pool(name="pb", bufs=2) as pb,
            tc.tile_pool(name="small", bufs=3) as spool,
            tc.tile_pool(name="psum", bufs=2, space="PSUM") as ppool,
        ):
            table = dpool.tile([n_tab, F], F32)

            wsb_sb = cpool.tile([F + 1, F], F32)
            nc.sync.dma_start(out=wsb_sb[:], in_=Wsb[:])
            wrb_sb = cpool.tile([F + 1, F], F32)
            nc.sync.dma_start(out=wrb_sb[:], in_=Wrb[:])
            awb_sb = cpool.tile([128, F], F32)
            nc.sync.dma_start(out=awb_sb[:], in_=awb[:])
            idx_sb = cpool.tile([128, S], I32)
            nc.sync.dma_start(out=idx_sb[:], in_=idxp[:])
            mask_sb = cpool.tile([128, S], F32)
            nc.sync.dma_start(out=mask_sb[:], in_=maskp[:])
            r_sb = cpool.tile([128, tiles * F], F32)
            awh_sb = cpool.tile([128, F], F16)
            nc.vector.tensor_copy(awh_sb[:], awb_sb[:])

            if VARIANT == "empty":
                ot0 = spool.tile([128, F], F32, tag="ot")
                nc.vector.tensor_copy(ot0[:], awb_sb[:])
                for t in range(tiles):
                    nc.sync.dma_start(out=outp[t * 128:(t + 1) * 128, :], in_=ot0[:])
            # phase 1a: r_proj for local nodes, resident in SBUF
            for t in range(tiles if VARIANT != "empty" else 0):
                xt = xpool.tile([F + 1, 128], F32, tag="xl")
                nc.sync.dma_start(out=xt[:], in_=xlT[:, t * 128:(t + 1) * 128])
                ps = ppool.tile([128, F], F32, tag="psr")
                nc.tensor.matmul(ps[:], lhsT=xt[:], rhs=wrb_sb[:],
                                 start=True, stop=True)
                nc.scalar.copy(r_sb[:, t * F:(t + 1) * F], ps[:])

            # phase 1b: s_proj table in HBM
            for g in range(n_grp if VARIANT != "empty" else 0):
                xg = xpool.tile([F + 1, 512], F32, tag="xg")
                nc.sync.dma_start(out=xg[:], in_=xT[:, g * 512:(g + 1) * 512])
                ps = ppool.tile([128, 4 * F], F32, tag="pss")
                for j in range(4):
                    nc.tensor.matmul(
                        ps[:, j * F:(j + 1) * F],
                        lhsT=xg[:, j * 128:(j + 1) * 128],
                        rhs=wsb_sb[:], start=True, stop=True)
                sg = xpool.tile([128, 4 * F], F32, tag="sg")
                nc.vector.tensor_copy(sg[:], ps[:])
                wr_pat = "(p j) c -> p j c" if TAU_WRITE else "(j p) c -> p j c"
                nc.sync.dma_start(
                    out=table[g * 512:(g + 1) * 512, :].rearrange(
                        wr_pat, p=128),
                    in_=sg[:].rearrange("p (j c) -> p j c", j=4))

            # phase 2: per-tile gather + edge math
            if VARIANT == "phase1_only":
                for t in range(tiles):
                    nc.sync.dma_start(out=outp[t * 128:(t + 1) * 128, :],
                                      in_=r_sb[:, t * F:(t + 1) * F])
            for t in range(tiles if VARIANT in ("full", "gather_only", "compute_only") else 0):
                Dt = D_t[t]
                off = OFF[t]
                KC = Dt * F
                se = pse.tile([128, KC], F32, tag="se")
                if VARIANT != "compute_only":
                    for k in range(Dt):
                        nc.gpsimd.indirect_dma_start(
                            out=se[:, k * F:(k + 1) * F],
                            out_offset=None,
                            in_=table[:],
                            in_offset=IndirectOffsetOnAxis(
                                ap=idx_sb[:, off + k:off + k + 1], axis=0),
                        )
                else:
                    nc.vector.tensor_copy(se[:, :F], r_sb[:, t * F:(t + 1) * F])
                if VARIANT == "gather_only":
                    nc.sync.dma_start(out=outp[t * 128:(t + 1) * 128, :],
                                      in_=se[:, :F])
                    continue
                re_b = r_sb[:, t * F:(t + 1) * F][:, None, :].to_broadcast(
                    [128, Dt, F])
                FD = F16 if FP16_MISH else F32
                z = pz.tile([128, KC], FD, tag="z")
                nc.vector.tensor_tensor(
                    out=z[:].rearrange("p (k c) -> p k c", c=F),
                    in0=se[:].rearrange("p (k c) -> p k c", c=F),
                    in1=re_b, op=ALU.add)
                # mish(z) = z * tanh(softplus(z)) = z * (1 - 2/((e^z+1)^2+1))
                # (no Mish LUT in this build; Exp+Square share one table set;
                #  fp16 overflow in (e^z+1)^2 yields inf -> rcp 0 -> m = z,
                #  which is the correct mish asymptote)
                et = pa.tile([128, KC], FD, tag="A")
                nc.scalar.activation(et[:], z[:], AT.Exp)
                q = pb.tile([128, KC], FD, tag="B")
                nc.scalar.activation(q[:], et[:], AT.Square, bias=1.0)
                den_m = pa.tile([128, KC], FD, tag="A")
                nc.vector.tensor_scalar_add(den_m[:], in0=q[:], scalar1=1.0)
                rcp_m = pb.tile([128, KC], FD, tag="B")
                with nc.allow_low_precision(reason="fp16 mish factor"):
                    nc.vector.reciprocal(rcp_m[:], den_m[:])
                zr = pa.tile([128, KC], FD, tag="A")
                nc.vector.tensor_tensor(out=zr[:], in0=z[:], in1=rcp_m[:],
                                        op=ALU.mult)
                m = pb.tile([128, KC], FD, tag="B")
                nc.vector.scalar_tensor_tensor(
                    out=m[:], in0=zr[:], scalar=-2.0, in1=z[:],
                    op0=ALU.mult, op1=ALU.add)
                aw_b = (awh_sb if FP16_MISH else awb_sb)[:][:, None, :].to_broadcast(
                    [128, Dt, F])
                mw = pa.tile([128, KC], FD, tag="A")
                nc.vector.tensor_tensor(
                    out=mw[:].rearrange("p (k c) -> p k c", c=F),
                    in0=m[:].rearrange("p (k c) -> p k c", c=F),
                    in1=aw_b, op=ALU.mult)
                logits = spool.tile([128, Dt * H], F32, tag="logits")
                nc.vector.tensor_reduce(
                    out=logits[:],
                    in_=mw[:].rearrange("p (k h d) -> p k h d", h=H, d=HD),
                    axis=mybir.AxisListType.X, op=ALU.add)
                # ab cancels in the softmax (constant shift) -- skip it
                ex = spool.tile([128, Dt * H], F32, tag="ex")
                nc.scalar.activation(ex[:], logits[:], AT.Exp)
                exm = spool.tile([128, Dt * H], F32, tag="exm")
                mask_b = mask_sb[:, off:off + Dt][:, :, None].to_broadcast(
                    [128, Dt, H])
                nc.vector.tensor_tensor(
                    out=exm[:].rearrange("p (k h) -> p k h", h=H),
                    in0=ex[:].rearrange("p (k h) -> p k h", h=H),
                    in1=mask_b, op=ALU.mult)
                den = spool.tile([128, H], F32, tag="den")
                nc.vector.tensor_reduce(
                    out=den[:],
                    in_=exm[:].rearrange("p (k h) -> p h k", h=H),
                    axis=mybir.AxisListType.X, op=ALU.add)
                rec = spool.tile([128, H], F32, tag="rec")
                nc.vector.reciprocal(rec[:], den[:])
                wse = pb.tile([128, KC], F32, tag="B")
                exm_b = exm[:].rearrange(
                    "p (k h) -> p k h", h=H)[:, :, :, None].to_broadcast(
                    [128, Dt, H, HD])
                nc.vector.tensor_tensor(
                    out=wse[:].rearrange("p (k h d) -> p k h d", h=H, d=HD),
                    in0=se[:].rearrange("p (k h d) -> p k h d", h=H, d=HD),
                    in1=exm_b, op=ALU.mult)
                num = spool.tile([128, F], F32, tag="num")
                nc.vector.tensor_reduce(
                    out=num[:],
                    in_=wse[:].rearrange("p (k c) -> p c k", c=F),
                    axis=mybir.AxisListType.X, op=ALU.add)
                ot = spool.tile([128, F], F32, tag="ot")
                rec_b = rec[:][:, :, None].to_broadcast([128, H, HD])
                nc.vector.tensor_tensor(
                    out=ot[:].rearrange("p (h d) -> p h d", h=H),
                    in0=num[:].rearrange("p (h d) -> p h d", h=H),
                    in1=rec_b, op=ALU.mult)
                nc.sync.dma_start(out=outp[t * 128:(t + 1) * 128, :], in_=ot[:])

    return nc


def kernel(x, Ws, bs, Wr, br, aw, ab, senders, receivers):
    x = np.asarray(x, np.float32)
    senders = np.asarray(senders, np.int32)
    receivers = np.asarray(receivers, np.int32)
    ins, meta = _host_prep(x, np.asarray(Ws), np.asarray(bs), np.asarray(Wr),
                           np.asarray(br), np.asarray(aw), np.asarray(ab),
                           senders, receivers)
    nc = _build_program(meta)
    if not nc.is_finalized():
        nc.finalize()
    in_maps = []
    for c in range(NC_CORES):
        in_maps.append({
            "xT": ins["xT"],
            "xlT": ins["xlT"][c],
            "Wsb": ins["Wsb"],
            "Wrb": ins["Wrb"],
            "awb": ins["awb"],
            "idx": ins["idx"][c],
            "mask": ins["mask"][c],
        })
    res = run_bass_kernel_spmd(nc, in_maps, core_ids=list(range(NC_CORES)))
    N = x.shape[0]
    order = meta["order"]
    out_full = np.zeros((N, F), dtype=np.float32)
    rows_per_core = -(-N // NC_CORES)
    for c in range(NC_CORES):
        rows = order[c::NC_CORES]
        out_full[rows] = res.results[c]["out"][: len(rows)]
    return out_full
